# revision 14
# baseline (speedup 1.0000x reference)
"""Trainium2 Bass kernel for nn_MistralAttention_KVmix (v2).

Decode-step (Q=1) Mistral GQA attention with a mixed-precision KV cache:
the oldest 7168 positions of K are fake-quantized (2-bit, group=32 along
seq per d-row) and of V (2-bit, group=32 along head-dim per position);
the last 1025 positions stay fp32.  RoPE on the new token, softmax over
8193 positions, output projection.

Sharding: tensor-parallel over the 8 KV heads (1 per NeuronCore), the 4
matching query heads ride along.  hidden_states replicated; o_proj
partial sums are summed across cores on the host.

v2 layout strategy (vs v1):
  - K^T is pre-transposed on the HOST ([B, D, S] fp32) -> no PE
    transposes for K, one big contiguous DMA per half.
  - V is tile-shuffled on the host so partition p holds s = 128t+p ->
    PV tiles line up with the p^T layout the score transposes produce.
  - Weights/hidden pre-shuffled + cast fp16 on host; all matmul
    operands fp16 (KV cache data flows through the quant path and is
    emitted as fp16 z-values).
  - K/V ship fp32: the reference's round() decisions are numerically
    too sensitive for 16-bit inputs (validated: bf16 -> 9e-2 rel err).
  - Quant transform: P1 y=x-mn, P2 t=y*inv (in-place, DVE for K /
    GpSimd for V), P3 r'=t+2^23 (ScalarE bias-add, fp32 RNE == round),
    P4 z=(r'-2^23)*sc -> fp16 (STT).  Group stats via tensor_reduce.
  - scores/PV: q (fp16) stationary, z/mn-broadcast rhs streams; exp on
    ScalarE with row-sum accumulation; 1/sum folded in after PV.
"""

import os
import sys

import numpy as np

for _p in ("/opt/trn_rl_repo",):
    if os.path.isdir(_p) and _p not in sys.path:
        sys.path.insert(0, _p)

import concourse.bass as bass
import concourse.mybir as mybir
import concourse.tile as tile
from concourse.bass_utils import run_bass_kernel_spmd

F32 = mybir.dt.float32
F16 = mybir.dt.float16
AX = mybir.AxisListType
OP = mybir.AluOpType
ACTF = mybir.ActivationFunctionType

B = 4
NH = 4          # query heads per core
D = 128
S = 8192
NQ = 7168       # quantized prefix length (both K and V)
NQT = 56        # NQ / 128 V s-tiles
NG = 224        # groups per d-row (K) / per s-row*4 (V)
MAGIC = 8388608.0        # 2^23: fp32 (t + MAGIC) does RNE-to-integer
INV_SQRT_D = float(1.0 / np.sqrt(np.float32(D)))
C1 = 6.28125             # Cody-Waite 2*pi split, exact in fp32
C2 = float(np.float32(2.0 * np.pi - 6.28125))
INV_2PI = float(np.float32(1.0 / (2.0 * np.pi)))
RMAGIC = 12582912.0      # 1.5*2^23 for range reduction rounding


def _bc(ap, axis, n):
    """Insert a stride-0 dim of size n at position `axis`."""
    shape = list(ap.shape)
    shape.insert(axis, n)
    return ap.unsqueeze(axis).to_broadcast(tuple(shape))


def _split_multi_waits(nc):
    """The walrus build in this container encodes at most ONE semaphore wait
    per TPB instruction ("Too many sync wait commands").  Tile's sem pass
    emits several.  Split: for each instruction with N>1 waits, insert N-1
    same-engine ENGINE_NOPs before it, each carrying one wait."""
    for f in nc.m.functions:
        blocks = list(f.blocks)
        for blk in blocks:
            live = blk.instructions
            orig = list(live)
            new = []
            changed = False
            for inst in orig:
                si = inst.sync_info
                waits = list(si.on_wait) if (si and si.on_wait) else []
                if len(waits) > 1 and inst.engine != mybir.EngineType.Unassigned:
                    eng = nc.engines[inst.engine]
                    for w in waits[:-1]:
                        nop = eng.drain().ins
                        for b2 in f.blocks:
                            l2 = b2.instructions
                            if l2 and l2[-1] is nop:
                                l2.pop()
                                break
                        nop.sync_info = mybir.SyncInfo(on_wait=[w],
                                                       on_update=[])
                        new.append(nop)
                    inst.sync_info = mybir.SyncInfo(
                        on_wait=[waits[-1]],
                        on_update=list(si.on_update or []))
                    changed = True
                new.append(inst)
            if changed:
                live[:] = new


def build_nc():
    nc = bass.Bass()

    hT = nc.declare_dram_parameter("hT", [128, 32, B], F16, isOutput=False)
    kT = nc.declare_dram_parameter("kT", [B, 128, S], F32, isOutput=False)
    vsh = nc.declare_dram_parameter("vsh", [B, 128, S], F32, isOutput=False)
    wq = nc.declare_dram_parameter("wq", [128, 32, NH * D], F16, isOutput=False)
    wk = nc.declare_dram_parameter("wk", [128, 32, D], F16, isOutput=False)
    wv = nc.declare_dram_parameter("wv", [128, 32, D], F16, isOutput=False)
    wo = nc.declare_dram_parameter("wo", [128, NH, 4096], F16, isOutput=False)
    pos = nc.declare_dram_parameter("pos", [1, B], F32, isOutput=False)
    ident = nc.declare_dram_parameter("ident", [128, 128], F32, isOutput=False)
    ident16 = nc.declare_dram_parameter("ident16", [128, 128], F16,
                                        isOutput=False)
    invf = nc.declare_dram_parameter("invf", [128, 1], F32, isOutput=False)
    sgn = nc.declare_dram_parameter("sgn", [128, 1], F32, isOutput=False)
    out_d = nc.declare_dram_parameter("out", [B, 4096], F32, isOutput=True)

    with tile.TileContext(nc) as tc:
        _emit(nc, tc, hT, kT, vsh, wq, wk, wv, wo, pos, ident, ident16, invf,
              sgn, out_d)
    _split_multi_waits(nc)
    return nc


def _emit(nc, tc, hT_d, kT_d, vsh_d, wq_d, wk_d, wv_d, wo_d, pos_d, ident_d,
          ident16_d, invf_d, sgn_d, out_d):
    from contextlib import ExitStack

    with ExitStack() as ctx:
        ec = ctx.enter_context
        singles = ec(tc.tile_pool(name="singles", bufs=1))
        kpool = ec(tc.tile_pool(name="kpool", bufs=2))
        vpool = ec(tc.tile_pool(name="vpool", bufs=2))
        zkp = ec(tc.tile_pool(name="zkp", bufs=2))
        zvp = ec(tc.tile_pool(name="zvp", bufs=2))
        stats = ec(tc.tile_pool(name="stats", bufs=3))
        wpool = ec(tc.tile_pool(name="wpool", bufs=2))
        ptp = ec(tc.tile_pool(name="ptp", bufs=3))
        misc = ec(tc.tile_pool(name="misc", bufs=3))
        ps_sc = ec(tc.tile_pool(name="ps_sc", bufs=2, space="PSUM"))
        ps_pt = ec(tc.tile_pool(name="ps_pt", bufs=2, space="PSUM"))
        ps_po = ec(tc.tile_pool(name="ps_po", bufs=1, space="PSUM"))
        ps_tp = ec(tc.tile_pool(name="ps_tp", bufs=2, space="PSUM"))

        # ---- constants -------------------------------------------------
        ident_sb = singles.tile([128, 128], F32)
        nc.sync.dma_start(out=ident_sb[:], in_=ident_d[:])
        ident16_sb = singles.tile([128, 128], F16)
        nc.sync.dma_start(out=ident16_sb[:], in_=ident16_d[:])
        invf_sb = singles.tile([128, 1], F32)
        nc.sync.dma_start(out=invf_sb[:], in_=invf_d[:])
        sgn_sb = singles.tile([128, 1], F32)
        nc.sync.dma_start(out=sgn_sb[:], in_=sgn_d[:])
        posr = singles.tile([128, B], F32)
        nc.sync.dma_start(out=posr[:], in_=pos_d[:].to_broadcast((128, B)))
        zerob = singles.tile([128, 1], F32)
        nc.vector.memset(zerob[:], 0.0)
        magicb = singles.tile([128, 1], F32)
        nc.vector.memset(magicb[:], MAGIC)
        hT = singles.tile([128, 32, B], F16)
        nc.sync.dma_start(out=hT[:], in_=hT_d[:])

        # ---- projections (q, k, v) -------------------------------------
        q_sb = singles.tile([B, NH * D], F32)
        k_sb = singles.tile([B, D], F32)
        vn_ps = None
        for w_d, n_cols, dst in ((wq_d, NH * D, q_sb), (wk_d, D, k_sb),
                                 (wv_d, D, None)):
            ps_p = ps_sc.tile([B, n_cols], F32, tag="sc")
            for kc in range(4):
                w_t = wpool.tile([128, 8, n_cols], F16, tag="wst")
                nc.sync.dma_start(out=w_t[:],
                                  in_=w_d[:, 8 * kc:8 * (kc + 1), :])
                for j in range(8):
                    k = 8 * kc + j
                    nc.tensor.matmul(ps_p[:], hT[:, k, :], w_t[:, j, :],
                                     start=(k == 0), stop=(k == 31))
            if dst is not None:
                nc.scalar.copy(dst[:], ps_p[:])
            else:
                vn_ps = ps_p
        v_new16 = singles.tile([B, D], F16)
        nc.scalar.copy(v_new16[:], vn_ps[:])
        # row-major copy of v_new onto partition 0 (PV tail rhs needs base 0)
        v_new_f = singles.tile([1, B, D], F16)
        for bb in range(B):
            nc.sync.dma_start(out=v_new_f[0:1, bb, :],
                              in_=v_new16[bb:bb + 1, :])

        # transpose q -> [128 d, 4 h, 4 b], k -> [128 d, 4 b]
        ps_qT = ps_tp.tile([128, NH * B], F32, tag="tp")
        for h in range(NH):
            nc.tensor.transpose(ps_qT[:, 4 * h:4 * h + 4],
                                q_sb[:, 128 * h:128 * (h + 1)],
                                ident_sb[0:B, 0:B])
        qT = singles.tile([128, NH, B], F32)
        nc.scalar.copy(qT[:].rearrange("p h b -> p (h b)"), ps_qT[:])
        ps_kT = ps_tp.tile([128, B], F32, tag="tp")
        nc.tensor.transpose(ps_kT[:], k_sb[:], ident_sb[0:B, 0:B])
        kTn = singles.tile([128, B], F32)
        nc.scalar.copy(kTn[:], ps_kT[:])

        # ---- RoPE ------------------------------------------------------
        fT = singles.tile([128, B], F32)
        nc.gpsimd.tensor_tensor(fT[:], posr[:], invf_sb[:].to_broadcast((128, B)), OP.mult)
        rk = singles.tile([128, B], F32)
        nc.gpsimd.tensor_scalar(rk[:], fT[:], INV_2PI, None, OP.mult)
        nc.gpsimd.tensor_scalar(rk[:], rk[:], RMAGIC, RMAGIC,
                                OP.add, OP.subtract)
        m1 = singles.tile([128, B], F32)
        nc.vector.scalar_tensor_tensor(m1[:], rk[:], -C1, fT[:],
                                       OP.mult, OP.add)
        nc.vector.scalar_tensor_tensor(m1[:], rk[:], -C2, m1[:],
                                       OP.mult, OP.add)
        sinT = singles.tile([128, B], F32)
        cosT = singles.tile([128, B], F32)
        nc.scalar.activation(sinT[:], m1[:], ACTF.Sin, bias=zerob[:])
        fc = singles.tile([128, B], F32)
        nc.gpsimd.tensor_scalar(fc[:], fT[:], float(np.pi / 2), None, OP.add)
        rkc = singles.tile([128, B], F32)
        nc.gpsimd.tensor_scalar(rkc[:], fc[:], INV_2PI, None, OP.mult)
        nc.gpsimd.tensor_scalar(rkc[:], rkc[:], RMAGIC, RMAGIC,
                                OP.add, OP.subtract)
        mc = singles.tile([128, B], F32)
        nc.vector.scalar_tensor_tensor(mc[:], rkc[:], -C1, fc[:],
                                       OP.mult, OP.add)
        nc.vector.scalar_tensor_tensor(mc[:], rkc[:], -C2, mc[:],
                                       OP.mult, OP.add)
        nc.scalar.activation(cosT[:], mc[:], ACTF.Sin, bias=zerob[:])
        nc.vector.tensor_scalar(sinT[:], sinT[:], sgn_sb[:], None, OP.mult)

        # rotate-half source: swap d halves
        qsw = singles.tile([128, NH, B], F32)
        nc.sync.dma_start(out=qsw[0:64], in_=qT[64:128])
        nc.sync.dma_start(out=qsw[64:128], in_=qT[0:64])
        ksw = singles.tile([128, B], F32)
        nc.sync.dma_start(out=ksw[0:64], in_=kTn[64:128])
        nc.sync.dma_start(out=ksw[64:128], in_=kTn[0:64])

        qR = singles.tile([128, NH, B], F32)
        nc.gpsimd.tensor_tensor(qR[:], qT[:], _bc(cosT[:], 1, NH), OP.mult)
        qs2 = singles.tile([128, NH, B], F32)
        nc.gpsimd.tensor_tensor(qs2[:], qsw[:], _bc(sinT[:], 1, NH), OP.mult)
        nc.gpsimd.tensor_tensor(qR[:], qR[:], qs2[:], OP.add)
        kR = singles.tile([128, B], F32)
        nc.gpsimd.tensor_tensor(kR[:], kTn[:], cosT[:], OP.mult)
        ks2 = singles.tile([128, B], F32)
        nc.gpsimd.tensor_tensor(ks2[:], ksw[:], sinT[:], OP.mult)
        nc.gpsimd.tensor_tensor(kR[:], kR[:], ks2[:], OP.add)
        qR16 = singles.tile([128, NH, B], F16)
        nc.scalar.copy(qR16[:], qR[:])
        kR16 = singles.tile([128, B], F16)
        nc.scalar.copy(kR16[:], kR[:])

        oT = singles.tile([128, NH, B], F16)

        for b in range(B):
            # ======== K path: halves of 4096 columns ========
            zK = zkp.tile([128, S], F16, tag="zK")
            mnK = stats.tile([128, NG], F32, tag="mnK")
            mxK = stats.tile([128, NG], F32, tag="mxK")
            scK = stats.tile([128, NG], F32, tag="scK")
            invK = stats.tile([128, NG], F32, tag="invK")
            scK16 = stats.tile([128, NG], F16, tag="scK16")
            mnK16 = stats.tile([128, NG], F16, tag="mnK16")
            for half in range(2):
                kzh = kpool.tile([128, 4096], F32, tag="kz")
                nc.sync.dma_start(
                    out=kzh[:],
                    in_=kT_d[b, :, 4096 * half:4096 * (half + 1)])
                nq_h = 4096 if half == 0 else NQ - 4096
                ng_h = nq_h // 32
                sl = slice(128 * half, 128 * half + ng_h)
                gv = kzh[:, 0:nq_h].rearrange("p (g e) -> p g e", e=32)
                nc.vector.tensor_reduce(mnK[:, sl], gv, axis=AX.X, op=OP.min)
                nc.vector.tensor_reduce(mxK[:, sl], gv, axis=AX.X, op=OP.max)
                # sc = (mx-mn)/3 fp32; inv = 1/sc; fp16 casts on ScalarE
                nc.vector.tensor_sub(scK[:, sl], mxK[:, sl], mnK[:, sl])
                nc.vector.reciprocal(invK[:, sl], scK[:, sl])
                # P1: y = x - mn (in place, GpSimd)
                nc.gpsimd.tensor_tensor(gv, gv, _bc(mnK[:, sl], 2, 32),
                                        OP.subtract)
                # P2: t = y * inv in [0,3] (in place, GpSimd)
                nc.gpsimd.tensor_tensor(gv, gv, _bc(invK[:, sl], 2, 32),
                                        OP.mult)
                # round on ScalarE: 3*t' + 2^23 (fp32 RNE), then -2^23
                nc.scalar.activation(kzh[:, 0:nq_h], kzh[:, 0:nq_h],
                                     ACTF.Copy, scale=3.0, bias=MAGIC)
                nc.scalar.activation(zK[:, 4096 * half:4096 * half + nq_h],
                                     kzh[:, 0:nq_h], ACTF.Copy, bias=-MAGIC)
                if half == 1:
                    # full-precision tail: fp32 -> fp16 cast
                    nc.scalar.copy(zK[:, NQ:S], kzh[:, nq_h:4096])
            nc.scalar.activation(scK16[:], scK[:], ACTF.Copy, scale=1.0 / 3.0)
            nc.scalar.copy(mnK16[:], mnK[:])
            # q' = q * sc per group: stationary for the folded score matmuls
            qp = stats.tile([128, NG, NH], F16, tag="qp")
            nc.gpsimd.tensor_tensor(
                qp[:], _bc(scK16[:], 2, NH),
                qR16[:, :, b].unsqueeze(1).to_broadcast((128, NG, NH)),
                OP.mult)

            # ======== V path: halves of 32 tiles ========
            zV = zvp.tile([128, 64, D], F16, tag="zV")
            mnV = stats.tile([128, NG], F32, tag="mnV")
            mxV = stats.tile([128, NG], F32, tag="mxV")
            scV = stats.tile([128, NG], F32, tag="scV")
            invV = stats.tile([128, NG], F32, tag="invV")
            scV16 = stats.tile([128, NG], F16, tag="scV16")
            mnV16 = stats.tile([128, NG], F16, tag="mnV16")
            mnVv = mnV[:].rearrange("p (t g) -> p t g", g=4)
            mxVv = mxV[:].rearrange("p (t g) -> p t g", g=4)
            invVv = invV[:].rearrange("p (t g) -> p t g", g=4)
            scV16v = scV16[:].rearrange("p (t g) -> p t g", g=4)
            mnV16v = mnV16[:].rearrange("p (t g) -> p t g", g=4)
            for half in range(2):
                vzh = vpool.tile([128, 4096], F32, tag="vz")
                nc.sync.dma_start(
                    out=vzh[:],
                    in_=vsh_d[b, :, 4096 * half:4096 * (half + 1)])
                nt_h = 32 if half == 0 else NQT - 32
                tsl = slice(32 * half, 32 * half + nt_h)
                vv = vzh[:, 0:128 * nt_h].rearrange(
                    "p (t g e) -> p t g e", g=4, e=32)
                nc.vector.tensor_reduce(mnVv[:, tsl, :], vv, axis=AX.X,
                                        op=OP.min)
                nc.vector.tensor_reduce(mxVv[:, tsl, :], vv, axis=AX.X,
                                        op=OP.max)
                gsl = slice(128 * half, 128 * half + 4 * nt_h)
                nc.vector.tensor_sub(scV[:, gsl], mxV[:, gsl], mnV[:, gsl])
                nc.vector.reciprocal(invV[:, gsl], scV[:, gsl])
                # P1/P2 on GpSimd (in place)
                nc.gpsimd.tensor_tensor(vv, vv, _bc(mnVv[:, tsl, :], 3, 32),
                                        OP.subtract)
                nc.gpsimd.tensor_tensor(vv, vv, _bc(invVv[:, tsl, :], 3, 32),
                                        OP.mult)
                # round on ScalarE: 3*t' + 2^23, then -2^23 -> fp16 r
                nc.scalar.activation(vzh[:, 0:128 * nt_h],
                                     vzh[:, 0:128 * nt_h],
                                     ACTF.Copy, scale=3.0, bias=MAGIC)
                nc.scalar.activation(
                    zV[:, tsl, :].rearrange("p t d -> p (t d)"),
                    vzh[:, 0:128 * nt_h], ACTF.Copy, bias=-MAGIC)
                if half == 1:
                    nc.scalar.copy(
                        zV[:, NQT:64, :].rearrange("p t d -> p (t d)"),
                        vzh[:, 128 * nt_h:4096])
            nc.scalar.activation(scV16[:], scV[:], ACTF.Copy, scale=1.0 / 3.0)
            nc.scalar.copy(mnV16[:], mnV[:])
            # ======== scores -> exp -> p^T ========
            qb = qR16[:, :, b]
            pT = ptp.tile([128, 65, NH], F16)
            sacc = misc.tile([NH, 17], F32, tag="sacc")
            for g4 in range(4):
                ppt = ps_pt.tile([128, 16, NH], F16, tag="pt")
                for cc in range(4):
                    c = 4 * g4 + cc
                    psc = ps_sc.tile([B, 512], F32, tag="sc")
                    if c < 14:
                        nc.tensor.matmul(psc[:], qb,
                                         _bc(mnK16[:, 16 * c:16 * (c + 1)],
                                             2, 32),
                                         start=True, stop=False)
                        for gg in range(16):
                            g = 16 * c + gg
                            nc.tensor.matmul(psc[:, 32 * gg:32 * (gg + 1)],
                                             qp[:, g, :],
                                             zK[:, 32 * g:32 * (g + 1)],
                                             start=False, stop=True)
                    else:
                        nc.tensor.matmul(psc[:], qb,
                                         zK[:, 512 * c:512 * (c + 1)],
                                         start=True, stop=True)
                    pexp = misc.tile([B, 512], F16, tag="pexp")
                    nc.scalar.activation(pexp[:], psc[:], ACTF.Exp,
                                         bias=zerob[0:B, :], scale=INV_SQRT_D,
                                         accum_out=sacc[:, c:c + 1])
                    for j in range(4):
                        nc.tensor.transpose(ppt[:, 4 * cc + j, :],
                                            pexp[:, 128 * j:128 * (j + 1)],
                                            ident16_sb[0:B, 0:B])
                nc.vector.tensor_copy(pT[:, 16 * g4:16 * (g4 + 1), :], ppt[:])
            # new-token column (s = 8192)
            psn = ps_sc.tile([B, 1], F32, tag="sc")
            nc.tensor.matmul(psn[:], qb, kR16[:, b:b + 1],
                             start=True, stop=True)
            pexp = misc.tile([B, 512], F16, tag="pexp")
            nc.scalar.activation(pexp[:, 0:1], psn[:], ACTF.Exp,
                                 bias=zerob[0:B, :], scale=INV_SQRT_D,
                                 accum_out=sacc[:, 16:17])
            pptn = ps_pt.tile([1, NH], F16, tag="pt")
            nc.tensor.transpose(pptn[:], pexp[:, 0:1], ident16_sb[0:B, 0:B])
            nc.vector.tensor_copy(pT[0:1, 64, :], pptn[:])
            stot = misc.tile([NH, 1], F32, tag="stot")
            nc.vector.tensor_reduce(stot[:], sacc[:], axis=AX.X, op=OP.add)
            rsc = misc.tile([NH, 1], F32, tag="rsc")
            nc.vector.reciprocal(rsc[:], stot[:])

            # p' = pT * sc_v per (tile, group): stationary for folded PV
            pp = ptp.tile([128, NQT, 4, NH], F16, tag="pp")
            nc.vector.tensor_tensor(
                pp[:], _bc(pT[:, 0:NQT, :], 2, 4),
                _bc(scV16v[:, 0:NQT, :], 3, NH), OP.mult)

            # ======== PV ========
            po = ps_po.tile([B, D], F32, tag="po")
            for t in range(NQT):
                nc.tensor.matmul(po[:], pT[:, t, :],
                                 _bc(mnV16v[:, t, :], 2, 32),
                                 start=(t == 0), stop=False)
                for g in range(4):
                    nc.tensor.matmul(po[:, 32 * g:32 * (g + 1)],
                                     pp[:, t, g, :],
                                     zV[:, t, 32 * g:32 * (g + 1)],
                                     start=False, stop=False)
            for t in range(NQT, 64):
                nc.tensor.matmul(po[:], pT[:, t, :], zV[:, t, :],
                                 start=False, stop=False)
            nc.tensor.matmul(po[:], pT[0:1, 64, :], v_new_f[0:1, b, :],
                             start=False, stop=True)
            ob = misc.tile([B, D], F16, tag="ob")
            nc.scalar.activation(ob[:], po[:], ACTF.Copy, scale=rsc[:])
            poT = ps_tp.tile([128, B], F16, tag="tp")
            nc.tensor.transpose(poT[:], ob[:], ident16_sb[0:B, 0:B])
            nc.vector.tensor_copy(oT[:, :, b], poT[:])

        # ---- o_proj ----------------------------------------------------
        for nch in range(8):
            wo_t = wpool.tile([128, NH, 512], F16, tag="wo")
            nc.sync.dma_start(out=wo_t[:],
                              in_=wo_d[:, :, 512 * nch:512 * (nch + 1)])
            pso = ps_sc.tile([B, 512], F32, tag="sc")
            for h in range(NH):
                nc.tensor.matmul(pso[:], oT[:, h, :], wo_t[:, h, :],
                                 start=(h == 0), stop=(h == NH - 1))
            outp = misc.tile([B, 512], F32, tag="outp")
            nc.scalar.copy(outp[:], pso[:])
            nc.sync.dma_start(out=out_d[:, 512 * nch:512 * (nch + 1)],
                              in_=outp[:])


# ----------------------------------------------------------------------
_NC = None


def _get_nc():
    global _NC
    if _NC is None:
        _NC = build_nc()
    return _NC


def _host_consts():
    ident = np.eye(128, dtype=np.float32)
    ident16 = np.eye(128, dtype=np.float16)
    inv_freq = (1.0 / (np.float32(10000.0) **
                       (np.arange(0, D, 2).astype(np.float32) /
                        np.float32(D))))
    invf = np.tile(inv_freq.astype(np.float32), 2).reshape(128, 1)
    sgn = np.concatenate([-np.ones(64, np.float32),
                          np.ones(64, np.float32)]).reshape(128, 1)
    return ident, ident16, invf, sgn


def _make_in_maps(inputs):
    hidden_states = np.asarray(inputs["hidden_states"], np.float32)
    key_past = np.asarray(inputs["key_past"], np.float32)
    value_past = np.asarray(inputs["value_past"], np.float32)
    wq = np.asarray(inputs["wq"], np.float32)
    wk = np.asarray(inputs["wk"], np.float32)
    wv = np.asarray(inputs["wv"], np.float32)
    wo = np.asarray(inputs["wo"], np.float32)
    position_ids = np.asarray(inputs["position_ids"])

    ident, ident16, invf, sgn = _host_consts()
    pos_f = position_ids.astype(np.float32).reshape(1, B)
    hid = hidden_states.reshape(B, 4096)
    hT = np.ascontiguousarray(
        hid.T.reshape(32, 128, B).transpose(1, 0, 2)).astype(np.float16)

    in_maps = []
    for c in range(8):
        kT = np.ascontiguousarray(key_past[:, c].transpose(0, 2, 1))
        vsh = np.ascontiguousarray(
            value_past[:, c].reshape(B, 64, 128, 128)
            .transpose(0, 2, 1, 3)).reshape(B, 128, S)
        wq_sh = np.ascontiguousarray(
            wq[:, 512 * c:512 * (c + 1)].reshape(32, 128, 512)
            .transpose(1, 0, 2)).astype(np.float16)
        wk_sh = np.ascontiguousarray(
            wk[:, 128 * c:128 * (c + 1)].reshape(32, 128, 128)
            .transpose(1, 0, 2)).astype(np.float16)
        wv_sh = np.ascontiguousarray(
            wv[:, 128 * c:128 * (c + 1)].reshape(32, 128, 128)
            .transpose(1, 0, 2)).astype(np.float16)
        wo_sh = np.ascontiguousarray(
            wo[512 * c:512 * (c + 1), :].reshape(4, 128, 4096)
            .transpose(1, 0, 2)).astype(np.float16)
        in_maps.append({
            "hT": hT,
            "kT": kT,
            "vsh": vsh,
            "wq": wq_sh,
            "wk": wk_sh,
            "wv": wv_sh,
            "wo": wo_sh,
            "pos": pos_f,
            "ident": ident,
            "ident16": ident16,
            "invf": invf,
            "sgn": sgn,
        })
    return in_maps


def kernel(hidden_states, key_past, value_past, wq, wk, wv, wo, position_ids,
           past_len):
    inputs = {
        "hidden_states": hidden_states, "key_past": key_past,
        "value_past": value_past, "wq": wq, "wk": wk, "wv": wv, "wo": wo,
        "position_ids": position_ids,
    }
    nc = _get_nc()
    in_maps = _make_in_maps(inputs)
    res = run_bass_kernel_spmd(nc, in_maps, list(range(8)))
    out = np.zeros((B, 4096), np.float32)
    for r in res.results:
        out = out + r["out"]
    return out.reshape(B, 1, 4096)
